# revision 2
# baseline (speedup 1.0000x reference)
"""DIN attention kernel for Trainium2 (8 NeuronCores, data-parallel batch).

Math (per sample, B=2048, L=200, D=128, H1=256, H2=128):
  att_in = [q, k, q-k, q*k];  h1 = prelu(att_in@W1 + b1, a1)
  h2 = prelu(h1@W2 + b2, a2); s = h2@W3 (+b3, dropped - softmax invariant)
  p = e*m / sum(e*m), e = exp(s)  (== reference's NEG+softmax+mask+renorm)
  out = p @ k

Fast path (mask-packed, duo-batched; used when every sample has <= 128
unmasked positions, which holds for this workload's Bernoulli(0.5) mask
over L=200 w.h.p.):
  - W1 folded on host: att@W1 = q@(W1a+W1c) + k@(W1b-W1c) + (q*k)@W1d.
  - mask packing: each sample keeps only its unmasked key positions,
    gathered front-packed and padded to 128 -> all per-position work
    (L1/L2 matmuls, activations, softmax, weighted sum) shrinks by
    ~200/128 and fits one 128-deep chunk. Padded positions carry mask=0
    and zero keys, so the math is identical to the unpacked kernel.
  - q-part qw = (W1a+W1c)^T q + b1 precomputed once per core on the PE.
    For h1 chunk 0 it enters as the per-sample bias of the Act-engine
    prelu; for chunk 1 it is accumulated into PSUM by a K=4 "selector"
    matmul (stationary = 4 samples' qw rows in duo-major form, moving =
    0/1 block mask), then drained by DVE copy + prelu (max(a*x, x)).
  - duo batching: 4 samples (2 pairs) per iteration -> 512-col matmuls
    (the ~190ns fixed cost per matmul amortizes), one [128,512]
    L2-prelu call, software-pipelined 2 deep so Act/DVE latency hides
    under the next duos' L1 matmuls.
  - scores via "masked w3" stationary (w3 in column p of a [128,32]
    stationary) accumulate pair rows into psum [32,256]; batched masked
    softmax on pair rows (no max-subtraction; scores are O(8));
    probs transposed back by PE; per-sample 128-deep matvec weighted
    sum; output written column-form [128, bc], host transposes back.

Fallback path (any sample > 128 unmasked): unpacked L=200 variant of the
same algorithm (pair-granular, l split 128+72).
"""

import sys

sys.path.insert(0, "/opt/trn_rl_repo")

import numpy as np

import concourse.bass as bass
import concourse.bacc as bacc
import concourse.mybir as mybir
import concourse.tile as tile
from concourse import masks
from concourse.bass_utils import run_bass_kernel_spmd

F32 = mybir.dt.float32
F16 = mybir.dt.float16
ALU = mybir.AluOpType
ACTF = mybir.ActivationFunctionType

B, L, D = 2048, 200, 128
H1, H2 = 256, 128
NCORES = 8
BLK = 64            # samples per block
NPAIR = BLK // 2    # 32 pairs -> 16 duos per block
NDUO = BLK // 4
L0, L1R = 128, 72   # l split for the unpacked fallback

_CACHE = {}


# ---------------------------------------------------------------------------
# fast path: mask-packed, duo-batched
# ---------------------------------------------------------------------------

def _build_packed(bc, a1v, a2v):
    lp = 128
    nblk = bc // BLK
    nc = bacc.Bacc("TRN2", target_bir_lowering=False, debug=False,
                   num_devices=NCORES)

    def din(name, shape, dt=F16):
        return nc.dram_tensor(name, shape, dt, kind="ExternalInput").ap()

    kaP_d = din("kaP", [nblk, 128, BLK * 128])   # [l, s*128+d]
    ktP_d = din("ktP", [nblk, D, BLK * lp])      # [d, s*lp+l]
    qT_d = din("qT", [D, bc], F32)
    qT16_d = din("qT16", [D, bc])
    maskp_d = din("maskp", [nblk, NPAIR, 2 * lp])
    w1bc_d = din("w1bc", [D, H1])
    w1d_d = din("w1d", [D, H1])
    w1ac_d = din("w1ac", [D, H1])
    w2a_d = din("w2a", [128, H2])
    w2b_d = din("w2b", [128, H2])
    w3m_d = din("w3m", [H2, NPAIR * 32])
    sel4_d = din("sel4", [4, 4 * lp])
    b1c_d = din("b1c", [128, 2], F32)
    b2c_d = din("b2c", [128, 1], F32)
    out_d = nc.dram_tensor("out", [D, bc], F32, kind="ExternalOutput").ap()

    with tile.TileContext(nc) as tc:
        with (
            tc.tile_pool(name="const", bufs=1) as cpool,
            tc.tile_pool(name="keys", bufs=2) as kpool,
            tc.tile_pool(name="work", bufs=3) as work,
            tc.tile_pool(name="tail", bufs=2) as tailp,
            tc.tile_pool(name="ps_h1", bufs=2, space="PSUM") as ps_h1,
            tc.tile_pool(name="ps_h2", bufs=2, space="PSUM") as ps_h2,
            tc.tile_pool(name="ps_sc", bufs=1, space="PSUM") as ps_sc,
            tc.tile_pool(name="ps_mc", bufs=1, space="PSUM") as ps_mc,
        ):
            ident = cpool.tile([128, 128], F32)
            masks.make_identity(nc, ident[:])

            w1bc = cpool.tile([D, H1], F16)
            w1d = cpool.tile([D, H1], F16)
            w1ac = cpool.tile([D, H1], F16)
            w2a = cpool.tile([128, H2], F16)
            w2b = cpool.tile([128, H2], F16)
            w3m = cpool.tile([H2, NPAIR * 32], F16)
            sel4 = cpool.tile([4, 4 * lp], F16)
            b1c = cpool.tile([128, 2], F32)
            b2c = cpool.tile([128, 1], F32)
            qT = cpool.tile([D, bc], F32)
            qT16 = cpool.tile([D, bc], F16)
            qw = cpool.tile([128, 2 * bc], F32)   # bias cols: jc*bc + s
            qw1r = cpool.tile([128, 256], F16)    # qw chunk1 rows
            qw1d = cpool.tile([4, (bc // 4) * 128], F16)  # duo-major rows
            nc.sync.dma_start(w1bc[:], w1bc_d[:])
            nc.sync.dma_start(w1d[:], w1d_d[:])
            nc.sync.dma_start(w1ac[:], w1ac_d[:])
            nc.sync.dma_start(w2a[:], w2a_d[:])
            nc.sync.dma_start(w2b[:], w2b_d[:])
            nc.sync.dma_start(w3m[:], w3m_d[:])
            nc.sync.dma_start(sel4[:], sel4_d[:])
            nc.sync.dma_start(b1c[:], b1c_d[:])
            nc.sync.dma_start(b2c[:], b2c_d[:])
            nc.sync.dma_start(qT[:], qT_d[:])
            nc.sync.dma_start(qT16[:], qT16_d[:])

            # one long-lived psum bank: preamble qw / tail probs^T + u
            misc = ps_mc.tile([128, 512], F32)

            # qw[h1, s] = (W1a+W1c)^T q + b1  (once per core)
            for jc in range(2):
                js = slice(jc * 128, (jc + 1) * 128)
                nc.tensor.matmul(misc[:, jc * 256:(jc + 1) * 256],
                                 w1ac[:, js], qT16[:], start=True, stop=True)
                nc.vector.tensor_scalar_add(
                    qw[:, jc * bc:(jc + 1) * bc],
                    misc[:, jc * 256:(jc + 1) * 256],
                    b1c[:, jc:jc + 1])

            # qw chunk-1 in duo-major row form for the selector matmul:
            # transpose qw[:, bc:2bc] -> rows s; a partition->free
            # rearrange must round-trip through DRAM (an SBUF AP cannot
            # step partitions in a free dim)
            for t in range(bc // 128):
                nc.tensor.matmul(misc[:, t * 128:(t + 1) * 128],
                                 qw[:, bc + t * 128:bc + (t + 1) * 128],
                                 ident[:], is_transpose=True)
            nc.vector.tensor_copy(qw1r[:], misc[:, 0:256])
            qwsc = nc.dram_tensor("qwscratch", [128, 256], F16,
                                  kind="Internal").ap()
            nc.sync.dma_start(qwsc[:], qw1r[:])
            nc.sync.dma_start(
                qw1d[:].rearrange("i (g d h) -> i g d h", g=bc // 128, h=128),
                qwsc.rearrange("(d i) (g h) -> i g d h", i=4, h=128))

            for ib in range(nblk):
                s0 = ib * BLK

                ka = kpool.tile([128, BLK * 128], F16, tag="ka")
                kt = kpool.tile([D, BLK * lp], F16, tag="kt")
                mb = kpool.tile([NPAIR, 2 * lp], F16, tag="mb")
                nc.sync.dma_start(ka[:], kaP_d[ib])
                nc.sync.dma_start(kt[:], ktP_d[ib])
                nc.sync.dma_start(mb[:], maskp_d[ib])

                scps = ps_sc.tile([NPAIR, 2 * lp], F32, tag="scps")

                # duo-level software pipeline, depth 2:
                #   stage A(d): pt, L1 matmuls (+selector), prelu1
                #   stage B(d-2): L2, prelu2, score matmuls
                pipe = []
                for dd in range(NDUO + 2):
                    if dd < NDUO:
                        g = s0 + 4 * dd           # first global sample
                        sA = 4 * dd               # block-local

                        pt = work.tile([D, 4 * lp], F16, tag="pt")
                        for qs in range(4):
                            nc.vector.tensor_scalar_mul(
                                pt[:, qs * lp:(qs + 1) * lp],
                                kt[:, (sA + qs) * lp:(sA + qs + 1) * lp],
                                qT[:, g + qs:g + qs + 1])

                        h1p = ps_h1.tile([128, 1024], F32, tag="h1p")
                        ktv = kt[:, sA * lp:(sA + 4) * lp]
                        # chunk 0
                        nc.tensor.matmul(h1p[:, 0:512], w1bc[:, 0:128],
                                         ktv, start=True, stop=False)
                        nc.tensor.matmul(h1p[:, 0:512], w1d[:, 0:128],
                                         pt[:], start=False, stop=True)
                        # chunk 1 (+ qw via selector matmul)
                        nc.tensor.matmul(h1p[:, 512:1024], w1bc[:, 128:256],
                                         ktv, start=True, stop=False)
                        nc.tensor.matmul(h1p[:, 512:1024], w1d[:, 128:256],
                                         pt[:], start=False, stop=False)
                        gd = s0 // 4 + dd
                        nc.tensor.matmul(h1p[:, 512:1024],
                                         qw1d[:, gd * 128:(gd + 1) * 128],
                                         sel4[:], start=False, stop=True)

                        h1s = work.tile([128, 1024], F16, tag="h1s")
                        # chunk 0: Act prelu with per-sample qw bias
                        for qs in range(4):
                            nc.scalar.activation(
                                h1s[:, qs * lp:(qs + 1) * lp],
                                h1p[:, qs * lp:(qs + 1) * lp],
                                ACTF.Prelu,
                                bias=qw[:, g + qs:g + qs + 1],
                                scale=1.0, alpha=float(a1v))
                        # chunk 1: DVE drain + prelu (qw already in psum)
                        t1 = work.tile([128, 512], F16, tag="t1")
                        nc.vector.tensor_copy(t1[:], h1p[:, 512:1024])
                        nc.vector.scalar_tensor_tensor(
                            h1s[:, 512:1024], t1[:], float(a1v), t1[:],
                            op0=ALU.mult, op1=ALU.max)
                        pipe.append((h1s, dd))

                    if dd >= 2:
                        ph1s, pd = pipe.pop(0)
                        h2d = ps_h2.tile([128, 512], F32, tag="h2d")
                        nc.tensor.matmul(h2d[:], w2a[:], ph1s[:, 0:512],
                                         start=True, stop=False)
                        nc.tensor.matmul(h2d[:], w2b[:], ph1s[:, 512:1024],
                                         start=False, stop=True)
                        h2s = work.tile([128, 512], F16, tag="h2s")
                        nc.scalar.activation(
                            h2s[:], h2d[:], ACTF.Prelu,
                            bias=b2c[:, 0:1], scale=1.0, alpha=float(a2v))
                        for half in range(2):
                            qp = 2 * pd + half
                            nc.tensor.matmul(
                                scps[:], w3m[:, qp * 32:(qp + 1) * 32],
                                h2s[:, half * 256:half * 256 + 256],
                                start=(qp == 0), stop=(qp == NPAIR - 1))

                # ---- block tail ----
                e = tailp.tile([NPAIR, 2 * lp], F32, tag="e")
                nc.scalar.activation(e[:], scps[:], ACTF.Exp, scale=1.0)
                e2 = tailp.tile([NPAIR, 2 * lp], F32, tag="e2")
                den = tailp.tile([NPAIR, 2], F32, tag="den")
                for si in range(2):
                    nc.vector.scalar_tensor_tensor(
                        e2[:, si * lp:(si + 1) * lp],
                        e[:, si * lp:(si + 1) * lp], 1.0,
                        mb[:, si * lp:(si + 1) * lp],
                        op0=ALU.mult, op1=ALU.mult,
                        accum_out=den[:, si:si + 1])
                rec = tailp.tile([NPAIR, 2], F32, tag="rec")
                nc.vector.tensor_scalar_max(den[:], den[:], 1e-12)
                nc.vector.reciprocal(rec[:], den[:])
                probs = tailp.tile([NPAIR, 2 * lp], F32, tag="probs")
                for si in range(2):
                    nc.vector.tensor_scalar_mul(
                        probs[:, si * lp:(si + 1) * lp],
                        e2[:, si * lp:(si + 1) * lp], rec[:, si:si + 1])

                # transpose probs to columns in misc[:, 0:64]
                for si in range(2):
                    nc.tensor.matmul(
                        misc[0:128, si * 32:si * 32 + 32],
                        probs[:, si * lp:si * lp + 128],
                        ident[0:NPAIR, 0:NPAIR], is_transpose=True)
                pT16 = tailp.tile([128, 64], F16, tag="pT16")
                nc.vector.tensor_copy(pT16[:], misc[:, 0:64])

                # weighted sum: u columns [128(d), 64(s)] in misc[:, 256:320]
                for sl in range(BLK):
                    ph, si = sl // 2, sl % 2
                    nc.tensor.matmul(
                        misc[:, 256 + sl:257 + sl],
                        ka[:, sl * 128:(sl + 1) * 128],
                        pT16[:, si * 32 + ph:si * 32 + ph + 1],
                        start=True, stop=True)

                usb = tailp.tile([128, BLK], F32, tag="usb")
                nc.vector.tensor_copy(usb[:], misc[:, 256:256 + BLK])
                nc.sync.dma_start(out_d[:, s0:s0 + BLK], usb[:])

    nc.compile()
    return nc


def _prep_packed(query, keys, mask, W1, b1, a1, W2, b2, a2, W3):
    lp = 128
    W1 = np.asarray(W1, np.float32)
    w1a, w1b, w1c, w1d = W1[0:128], W1[128:256], W1[256:384], W1[384:512]
    W2 = np.asarray(W2, np.float32)
    W3 = np.asarray(W3, np.float32)

    w3m = np.zeros((H2, NPAIR, 32), np.float16)
    for p in range(NPAIR):
        w3m[:, p, p] = W3[:, 0]

    sel4 = np.zeros((4, 4 * lp), np.float16)
    for i in range(4):
        sel4[i, i * lp:(i + 1) * lp] = 1.0

    shared = dict(
        w1bc=np.ascontiguousarray((w1b - w1c).astype(np.float16)),
        w1d=np.ascontiguousarray(w1d.astype(np.float16)),
        w1ac=np.ascontiguousarray((w1a + w1c).astype(np.float16)),
        w2a=np.ascontiguousarray(W2[0:128].astype(np.float16)),
        w2b=np.ascontiguousarray(W2[128:256].astype(np.float16)),
        w3m=np.ascontiguousarray(w3m.reshape(H2, NPAIR * 32)),
        sel4=sel4,
        b1c=np.ascontiguousarray(np.asarray(b1, np.float32).reshape(2, 128).T),
        b2c=np.ascontiguousarray(np.asarray(b2, np.float32).reshape(128, 1)),
    )

    bc = query.shape[0] // NCORES
    nblk = bc // BLK
    maski = np.asarray(mask, np.int32)

    # pack unmasked positions to the front (stable keeps original order)
    order = np.argsort(maski == 0, axis=1, kind="stable")[:, :lp]
    keys16 = np.asarray(keys, np.float32).astype(np.float16)
    kp = np.take_along_axis(keys16, order[:, :, None], axis=1)
    mp = np.take_along_axis(maski, order, axis=1).astype(np.float16)
    kp = kp * mp[:, :, None]

    in_maps = []
    for c in range(NCORES):
        s = slice(c * bc, (c + 1) * bc)
        qs = np.asarray(query[s], np.float32)
        kpc = kp[s]
        m = dict(shared)
        m["kaP"] = np.ascontiguousarray(
            kpc.reshape(nblk, BLK, 128, 128)
            .transpose(0, 2, 1, 3).reshape(nblk, 128, BLK * 128))
        m["ktP"] = np.ascontiguousarray(
            kpc.transpose(0, 2, 1).reshape(nblk, BLK, 128, lp)
            .transpose(0, 2, 1, 3).reshape(nblk, 128, BLK * lp))
        m["qT"] = np.ascontiguousarray(qs.T)
        m["qT16"] = np.ascontiguousarray(qs.T.astype(np.float16))
        m["maskp"] = np.ascontiguousarray(
            mp[s].reshape(nblk, NPAIR, 2 * lp))
        in_maps.append(m)
    return in_maps


# ---------------------------------------------------------------------------
# fallback path: unpacked L=200 (used only if some sample has > 128
# unmasked positions)
# ---------------------------------------------------------------------------

def _build_unpacked(bc, a1v, a2v):
    nblk = bc // BLK
    nc = bacc.Bacc("TRN2", target_bir_lowering=False, debug=False,
                   num_devices=NCORES)

    def din(name, shape, dt=F16):
        return nc.dram_tensor(name, shape, dt, kind="ExternalInput").ap()

    kaH_d = din("kaH", [nblk, L0, BLK * 128])
    kbH_d = din("kbH", [nblk, L1R, BLK * 128])
    ktH_d = din("ktH", [nblk, D, BLK * L])
    qT_d = din("qT", [D, bc], F32)
    qT16_d = din("qT16", [D, bc])
    maskp_d = din("maskp", [nblk, NPAIR, 2 * L])
    w1bc_d = din("w1bc", [D, H1])
    w1d_d = din("w1d", [D, H1])
    w1ac_d = din("w1ac", [D, H1])
    w2a_d = din("w2a", [128, H2])
    w2b_d = din("w2b", [128, H2])
    w3m_d = din("w3m", [H2, NPAIR * 32])
    b1c_d = din("b1c", [128, 2], F32)
    b2c_d = din("b2c", [128, 1], F32)
    out_d = nc.dram_tensor("out", [D, bc], F32, kind="ExternalOutput").ap()

    with tile.TileContext(nc) as tc:
        with (
            tc.tile_pool(name="const", bufs=1) as cpool,
            tc.tile_pool(name="keys", bufs=2) as kpool,
            tc.tile_pool(name="work", bufs=2) as work,
            tc.tile_pool(name="tail", bufs=2) as tailp,
            tc.tile_pool(name="ps_h1", bufs=2, space="PSUM") as ps_h1,
            tc.tile_pool(name="ps_h2", bufs=2, space="PSUM") as ps_h2,
            tc.tile_pool(name="ps_sc", bufs=1, space="PSUM") as ps_sc,
            tc.tile_pool(name="ps_mc", bufs=1, space="PSUM") as ps_mc,
        ):
            ident = cpool.tile([128, 128], F32)
            masks.make_identity(nc, ident[:])

            w1bc = cpool.tile([D, H1], F16)
            w1d = cpool.tile([D, H1], F16)
            w1ac = cpool.tile([D, H1], F16)
            w2a = cpool.tile([128, H2], F16)
            w2b = cpool.tile([128, H2], F16)
            w3m = cpool.tile([H2, NPAIR * 32], F16)
            b1c = cpool.tile([128, 2], F32)
            b2c = cpool.tile([128, 1], F32)
            qT = cpool.tile([D, bc], F32)
            qT16 = cpool.tile([D, bc], F16)
            qw = cpool.tile([128, 2 * bc], F32)
            nc.sync.dma_start(w1bc[:], w1bc_d[:])
            nc.sync.dma_start(w1d[:], w1d_d[:])
            nc.sync.dma_start(w1ac[:], w1ac_d[:])
            nc.sync.dma_start(w2a[:], w2a_d[:])
            nc.sync.dma_start(w2b[:], w2b_d[:])
            nc.sync.dma_start(w3m[:], w3m_d[:])
            nc.sync.dma_start(b1c[:], b1c_d[:])
            nc.sync.dma_start(b2c[:], b2c_d[:])
            nc.sync.dma_start(qT[:], qT_d[:])
            nc.sync.dma_start(qT16[:], qT16_d[:])

            misc = ps_mc.tile([128, 512], F32)

            for jc in range(2):
                js = slice(jc * 128, (jc + 1) * 128)
                nc.tensor.matmul(misc[:, jc * 256:(jc + 1) * 256],
                                 w1ac[:, js], qT16[:], start=True, stop=True)
                nc.vector.tensor_scalar_add(
                    qw[:, jc * bc:(jc + 1) * bc],
                    misc[:, jc * 256:(jc + 1) * 256],
                    b1c[:, jc:jc + 1])

            for ib in range(nblk):
                s0 = ib * BLK

                ka = kpool.tile([L0, BLK * 128], F16, tag="ka")
                kb = kpool.tile([L1R, BLK * 128], F16, tag="kb")
                kt = kpool.tile([D, BLK * L], F16, tag="kt")
                mb = kpool.tile([NPAIR, 2 * L], F16, tag="mb")
                nc.sync.dma_start(ka[:], kaH_d[ib])
                nc.sync.dma_start(kb[:], kbH_d[ib])
                nc.sync.dma_start(kt[:], ktH_d[ib])
                nc.sync.dma_start(mb[:], maskp_d[ib])

                scps = ps_sc.tile([NPAIR, 2 * L], F32, tag="scps")

                for p in range(NPAIR):
                    sA = 2 * p
                    g = s0 + sA

                    pt = work.tile([D, 2 * L], F16, tag="pt")
                    for si in range(2):
                        nc.vector.tensor_scalar_mul(
                            pt[:, si * L:(si + 1) * L],
                            kt[:, (sA + si) * L:(sA + si + 1) * L],
                            qT[:, g + si:g + si + 1])

                    h1p = ps_h1.tile([128, 1024], F32, tag="h1p")
                    ktv = kt[:, sA * L:(sA + 2) * L]
                    for jc in range(2):
                        js = slice(jc * 128, (jc + 1) * 128)
                        o = h1p[:, jc * 512:jc * 512 + 2 * L]
                        nc.tensor.matmul(o, w1bc[:, js], ktv,
                                         start=True, stop=False)
                        nc.tensor.matmul(o, w1d[:, js], pt[:],
                                         start=False, stop=True)

                    h1s = work.tile([128, 4 * L], F16, tag="h1s")
                    t1 = work.tile([128, 2 * L], F16, tag="t1")
                    for si in range(2):
                        nc.scalar.activation(
                            h1s[:, si * L:(si + 1) * L],
                            h1p[:, si * L:(si + 1) * L],
                            ACTF.Prelu,
                            bias=qw[:, g + si:g + si + 1],
                            scale=1.0, alpha=float(a1v))
                    for si in range(2):
                        nc.vector.tensor_scalar_add(
                            t1[:, si * L:(si + 1) * L],
                            h1p[:, 512 + si * L:512 + (si + 1) * L],
                            qw[:, bc + g + si:bc + g + si + 1])
                    nc.vector.scalar_tensor_tensor(
                        h1s[:, 2 * L:4 * L], t1[:], float(a1v), t1[:],
                        op0=ALU.mult, op1=ALU.max)

                    h2p = ps_h2.tile([128, 2 * L], F32, tag="h2p")
                    nc.tensor.matmul(h2p[:], w2a[:], h1s[:, 0:2 * L],
                                     start=True, stop=False)
                    nc.tensor.matmul(h2p[:], w2b[:], h1s[:, 2 * L:4 * L],
                                     start=False, stop=True)

                    h2s = work.tile([128, 2 * L], F16, tag="h2s")
                    nc.scalar.activation(h2s[:], h2p[:], ACTF.Prelu,
                                         bias=b2c[:, 0:1], scale=1.0,
                                         alpha=float(a2v))

                    nc.tensor.matmul(scps[:], w3m[:, p * 32:(p + 1) * 32],
                                     h2s[:], start=(p == 0),
                                     stop=(p == NPAIR - 1))

                e = tailp.tile([NPAIR, 2 * L], F32, tag="e")
                nc.scalar.activation(e[:], scps[:], ACTF.Exp, scale=1.0)
                e2 = tailp.tile([NPAIR, 2 * L], F32, tag="e2")
                den = tailp.tile([NPAIR, 2], F32, tag="den")
                for si in range(2):
                    nc.vector.scalar_tensor_tensor(
                        e2[:, si * L:(si + 1) * L],
                        e[:, si * L:(si + 1) * L], 1.0,
                        mb[:, si * L:(si + 1) * L],
                        op0=ALU.mult, op1=ALU.mult,
                        accum_out=den[:, si:si + 1])
                rec = tailp.tile([NPAIR, 2], F32, tag="rec")
                nc.vector.tensor_scalar_max(den[:], den[:], 1e-12)
                nc.vector.reciprocal(rec[:], den[:])
                probs = tailp.tile([NPAIR, 2 * L], F32, tag="probs")
                for si in range(2):
                    nc.vector.tensor_scalar_mul(
                        probs[:, si * L:(si + 1) * L],
                        e2[:, si * L:(si + 1) * L], rec[:, si:si + 1])

                # transpose probs to columns: misc cols
                #   [0:32] even l0:128, [32:64] even l128:200,
                #   [64:96] odd l0:128, [96:128] odd l128:200
                nc.tensor.matmul(misc[0:128, 0:32], probs[:, 0:128],
                                 ident[0:NPAIR, 0:NPAIR], is_transpose=True)
                nc.tensor.matmul(misc[0:L1R, 32:64], probs[:, 128:200],
                                 ident[0:NPAIR, 0:NPAIR], is_transpose=True)
                nc.tensor.matmul(misc[0:128, 64:96], probs[:, 200:328],
                                 ident[0:NPAIR, 0:NPAIR], is_transpose=True)
                nc.tensor.matmul(misc[0:L1R, 96:128], probs[:, 328:400],
                                 ident[0:NPAIR, 0:NPAIR], is_transpose=True)
                pT16 = tailp.tile([128, 128], F16, tag="pT16")
                nc.vector.tensor_copy(pT16[:], misc[:, 0:128])

                for sl in range(BLK):
                    ph, si = sl // 2, sl % 2
                    c0 = si * 64 + ph
                    c1 = si * 64 + 32 + ph
                    uo = misc[:, 256 + sl:257 + sl]
                    nc.tensor.matmul(uo, ka[:, sl * 128:(sl + 1) * 128],
                                     pT16[0:L0, c0:c0 + 1],
                                     start=True, stop=False)
                    nc.tensor.matmul(uo, kb[:, sl * 128:(sl + 1) * 128],
                                     pT16[0:L1R, c1:c1 + 1],
                                     start=False, stop=True)

                usb = tailp.tile([128, BLK], F32, tag="usb")
                nc.vector.tensor_copy(usb[:], misc[:, 256:256 + BLK])
                nc.sync.dma_start(out_d[:, s0:s0 + BLK], usb[:])

    nc.compile()
    return nc


def _prep_unpacked(query, keys, mask, W1, b1, a1, W2, b2, a2, W3):
    W1 = np.asarray(W1, np.float32)
    w1a, w1b, w1c, w1d = W1[0:128], W1[128:256], W1[256:384], W1[384:512]
    W2 = np.asarray(W2, np.float32)
    W3 = np.asarray(W3, np.float32)

    w3m = np.zeros((H2, NPAIR, 32), np.float16)
    for p in range(NPAIR):
        w3m[:, p, p] = W3[:, 0]

    shared = dict(
        w1bc=np.ascontiguousarray((w1b - w1c).astype(np.float16)),
        w1d=np.ascontiguousarray(w1d.astype(np.float16)),
        w1ac=np.ascontiguousarray((w1a + w1c).astype(np.float16)),
        w2a=np.ascontiguousarray(W2[0:128].astype(np.float16)),
        w2b=np.ascontiguousarray(W2[128:256].astype(np.float16)),
        w3m=np.ascontiguousarray(w3m.reshape(H2, NPAIR * 32)),
        b1c=np.ascontiguousarray(np.asarray(b1, np.float32).reshape(2, 128).T),
        b2c=np.ascontiguousarray(np.asarray(b2, np.float32).reshape(128, 1)),
    )

    bc = query.shape[0] // NCORES
    nblk = bc // BLK
    keys16 = np.asarray(keys, np.float32).astype(np.float16)
    maskf = np.asarray(mask, np.int32).astype(np.float16)
    in_maps = []
    for c in range(NCORES):
        s = slice(c * bc, (c + 1) * bc)
        qs = np.asarray(query[s], np.float32)
        k16 = keys16[s]
        m = dict(shared)
        m["kaH"] = np.ascontiguousarray(
            k16[:, 0:L0, :].reshape(nblk, BLK, L0, 128)
            .transpose(0, 2, 1, 3).reshape(nblk, L0, BLK * 128))
        m["kbH"] = np.ascontiguousarray(
            k16[:, L0:L, :].reshape(nblk, BLK, L1R, 128)
            .transpose(0, 2, 1, 3).reshape(nblk, L1R, BLK * 128))
        m["ktH"] = np.ascontiguousarray(
            k16.transpose(0, 2, 1).reshape(nblk, BLK, 128, L)
            .transpose(0, 2, 1, 3).reshape(nblk, 128, BLK * L))
        m["qT"] = np.ascontiguousarray(qs.T)
        m["qT16"] = np.ascontiguousarray(qs.T.astype(np.float16))
        m["maskp"] = np.ascontiguousarray(
            maskf[s].reshape(nblk, NPAIR, 2 * L))
        in_maps.append(m)
    return in_maps


def kernel(query, keys, mask, W1, b1, a1, W2, b2, a2, W3, b3, trace=False):
    query = np.asarray(query, np.float32)
    a1v, a2v = float(np.asarray(a1).reshape(-1)[0]), float(
        np.asarray(a2).reshape(-1)[0])
    maski = np.asarray(mask, np.int32)
    packed = int((maski != 0).sum(axis=1).max()) <= 128
    bc = query.shape[0] // NCORES

    if packed:
        in_maps = _prep_packed(query, keys, mask, W1, b1, a1, W2, b2, a2, W3)
        key = ("p", bc, a1v, a2v)
        if key not in _CACHE:
            _CACHE[key] = _build_packed(bc, a1v, a2v)
    else:
        in_maps = _prep_unpacked(query, keys, mask, W1, b1, a1, W2, b2, a2,
                                 W3)
        key = ("u", bc, a1v, a2v)
        if key not in _CACHE:
            _CACHE[key] = _build_unpacked(bc, a1v, a2v)
    nc = _CACHE[key]

    res = run_bass_kernel_spmd(
        nc, in_maps, core_ids=list(range(NCORES)), trace=trace)
    out = np.concatenate(
        [res.results[c]["out"].T for c in range(NCORES)], axis=0)
    kernel.last_results = res
    return out.astype(np.float32)
